# revision 35
# baseline (speedup 1.0000x reference)
"""Windowed multi-head self-attention Bass kernel for Trainium2.

Shapes (hardcoded): input [64, 256, 1536] fp32 (packed qkv, 32 heads x 16 dim),
rel_bias_table [127, 32] fp32. Output [64, 256, 512] fp32.

Sharding: data-parallel over the window axis B=64 across 8 NeuronCores
(8 windows per core). Bias tables are preprocessed on host and replicated.

Per-core pipeline (window pairs, software-pipelined):
  - 1 DMA per window loads [128, 2, 1536] fp32 (pair-0 split qk/v so
    transposes start early).
  - PE-transposes q/k sections ([128,128] fp32 blocks) into borrowed
    scores-pool PSUM tiles; DVE copies PSUM->SBUF casting to bf16.
  - Stitched bf16 score matmuls, pair-parity double-buffered: stationary
    KD[:,pr,qd,w,mch,:] holds a 4-head kT quad (rows 0-63) + one-hot U
    rows (64-127, static); moving QS[:,pr,hp,g,w,:] holds qT_h at rows
    16*(h%4) (other q-rows zeroed once at init) + per-head rel-pos bias
    rows G'_h (static).  One matmul per (head, m-chunk) lands
    S^T + bias/scale in PSUM ([128,1024] tiles, 3 in flight).
  - exp split 22:10 per 32 units between ACT (exact exp, scale folded)
    and DVE (Schraudolph exp: bits = round(x*A + B) as uint16, bitcast
    bf16; +-3% rel err, common mode cancels in softmax norm).  GPSIMD
    cannot touch PSUM on TRN2, so Pool only does SBUF work (vb copies)
    and the SWDGE KD stitches.
  - PV reoriented: out[n-chunk, 17] = P^T-chunk.T @ v' (v plus ones
    column -> col 16 = softmax denominator).  17-wide outputs make PV
    ~8x cheaper on PE than the [17,256] orientation and kill the
    output-side transposes.
  - DVE reciprocal of denominators + broadcast-multiply write the
    normalized [n, h*16] output tile; 1 DMA stores each window.
"""

import numpy as np
from contextlib import ExitStack

import concourse.bass as bass
import concourse.bacc as bacc
import concourse.tile as tile
from concourse import mybir
from concourse.bass_utils import run_bass_kernel_spmd

F32 = mybir.dt.float32
F32R = mybir.dt.float32r
BF16 = mybir.dt.bfloat16
U16 = mybir.dt.uint16

NCORES = 8
B = 64
W = B // NCORES
N = 256
C = 1536
NH = 32
HD = 16
SCALE = float(NH) ** -0.5
EXP_A = 128.0 * 1.4426950408889634 * SCALE  # schraudolph mult
EXP_B = 16250.5                             # schraudolph add (calibrated)

# exp engine split: 64 instrs of [128,512] per pair; A=ACT, D=DVE, P=Pool
def _mk_exp_pattern(n_a=22, n_d=10, n_p=0):
    n = n_a + n_d + n_p
    slots = [None] * n
    for kind, cnt in (("A", n_a), ("D", n_d), ("P", n_p)):
        if cnt == 0:
            continue
        for k in range(cnt):
            i = int(k * n / cnt)
            while slots[i % n] is not None:
                i += 1
            slots[i % n] = kind
    return "".join(slots)

EXP_PATTERN = _mk_exp_pattern()
assert len(EXP_PATTERN) == 32


def _build_kernel_body(ctx, tc, out, inp, gbias, uhot, ident):
    nc = tc.nc

    singles = ctx.enter_context(tc.tile_pool(name="singles", bufs=1))
    inpool = ctx.enter_context(tc.tile_pool(name="inpool", bufs=2))
    tpool = ctx.enter_context(tc.tile_pool(name="tpool", bufs=2))
    vpool = ctx.enter_context(tc.tile_pool(name="vpool", bufs=2))
    ppool = ctx.enter_context(tc.tile_pool(name="ppool", bufs=6))
    opool = ctx.enter_context(tc.tile_pool(name="opool", bufs=2))
    rpool = ctx.enter_context(tc.tile_pool(name="rpool", bufs=4))
    ps_s = ctx.enter_context(tc.tile_pool(name="ps_s", bufs=3, space="PSUM"))
    ps_pv = ctx.enter_context(tc.tile_pool(name="ps_pv", bufs=1, space="PSUM"))

    # --- static tiles (DMAs for these are emitted after the first loads) ---
    id_t = singles.tile([128, 128], F32, tag="ident")

    # QS_all[p, pr, hp, g, w, n]: moving operands; rows 0-63 qT (dynamic),
    # rows 64-127 G'_h (static).  Head h = hp + 16*g; pr = pair parity.
    QS = singles.tile([128, 2, 16, 2, 2, 256], BF16, tag="qs", name="qs")
    # KD_all[p, pr, qd, w, mch, m]: stationary; rows 0-63 kT quad (dynamic),
    # rows 64-127 U one-hot (static).
    KD = singles.tile([128, 2, 8, 2, 2, 128], BF16, tag="kd", name="kd")

    def emit_init_a(pr):
        # zero the q-region once: stitches only ever write 16 of each
        # slot's 64 rows; the rest must stay zero for the stitched matmul
        qz = QS[0:64, pr, :, :, :, :]
        nc.scalar.memzero(qz[:, 0:8].rearrange("p h g w n -> p (h g w n)"))
        nc.gpsimd.memset(qz[:, 8:16], 0.0)
        if pr == 0:
            nc.sync.dma_start(out=id_t[:], in_=ident)
        nc.sync.dma_start(
            out=KD[64:128, pr, :, :, :, :].rearrange(
                "p q w c m -> p (q w c m)"),
            in_=uhot)
        nc.sync.dma_start(
            out=QS[64:128, pr, 0:4, :, :, :].rearrange(
                "p h g w n -> p h (g w n)"),
            in_=gbias[:, 0:4, :])

    def emit_init_b(pr):
        for q4 in range(1, 4):
            nc.sync.dma_start(
                out=QS[64:128, pr, 4 * q4:4 * q4 + 4, :, :, :].rearrange(
                    "p h g w n -> p h (g w n)"),
                in_=gbias[:, 4 * q4:4 * q4 + 4, :])

    NP = W // 2
    st = {}

    def emit_load(wp, part=None):
        # part None: both; "qk": cols 0-1024 only; "v": cols 1024-1536
        xin = st.get(wp, {}).get("xin")
        if xin is None:
            xin = {ws: inpool.tile([128, 2, C], F32, tag=f"xin{ws}",
                                   name=f"xin{ws}") for ws in range(2)}
            st[wp] = {"xin": xin}
        lo, hi = {"qk": (0, 1024), "v": (1024, C)}.get(part, (0, C))
        for ws in range(2):
            nc.sync.dma_start(
                out=xin[ws][:, :, lo:hi],
                in_=inp[2 * wp + ws].rearrange(
                    "(c p) f -> p c f", p=128)[:, :, lo:hi])

    trstate = {"tile": None, "slot": 0}

    def _tr_block(wp, sec, dst, cb):
        xin = st[wp]["xin"]
        for ch in range(2):
            if trstate["slot"] % 4 == 0:
                trstate["tile"] = ps_s.tile([128, 1024], F32, tag="scores",
                                            name="scores")
            sl = trstate["slot"] % 4
            trstate["slot"] += 1
            tr = trstate["tile"][:, sl * 256:(sl + 1) * 256]
            for ws in range(2):
                nc.tensor.transpose(
                    tr[:, ws * 128:(ws + 1) * 128],
                    xin[ws][:, ch, sec + cb * 128: sec + (cb + 1) * 128],
                    id_t[:],
                )
            nc.vector.tensor_copy(
                dst[:, cb, :, ch * 128:(ch + 1) * 128],
                tr.rearrange("p (w n) -> p w n", w=2),
            )

    def emit_prep(wp):
        # transposes + copies + stitch, ordered so stitch sources land first
        xin = st[wp]["xin"]
        qt = tpool.tile([128, 4, 2, 256], BF16, tag="qt", name="qt")
        kt = tpool.tile([128, 4, 2, 256], BF16, tag="kt", name="kt")
        st[wp]["qt"] = qt
        st[wp]["kt"] = kt
        for cbp in range(2):
            for cb in (cbp, cbp + 2):
                _tr_block(wp, 512, kt, cb)
            for u in range(2):
                emit_stitch_kd(wp, u, cbp)
            for cb in (cbp, cbp + 2):
                _tr_block(wp, 0, qt, cb)
            for j in range(8):
                emit_stitch_qs(wp, cbp, j)
        vb = {}
        for ws in range(2):
            for ch in range(2):
                t = vpool.tile([128, NH, 17], BF16, tag=f"vb{ws}{ch}",
                               name=f"vb{ws}{ch}")
                nc.gpsimd.tensor_copy(
                    t[:, :, 0:16],
                    xin[ws][:, ch, 1024:1536].rearrange(
                        "p (h d) -> p h d", d=16),
                )
                if wp < 2:
                    nc.gpsimd.memset(t[:, :, 16:17], 1.0)
                vb[(ws, ch)] = t
        st[wp]["vb"] = vb

    def emit_stitch_qs(wp, cb, j):
        # heads h=8*cb+j and h+16 in one DMA
        qt = st[wp]["qt"]
        t = j % 4
        nc.sync.dma_start(
            out=QS[16 * t:16 * t + 16, wp % 2, 8 * cb + j, :, :, :],
            in_=qt[16 * j:16 * j + 16, cb::2, :, :])

    def emit_stitch_kd(wp, u, cb2):
        # quads qd=2*cb2+u and qd+4 in one DMA
        kt = st[wp]["kt"]
        nc.gpsimd.dma_start(
            out=KD[0:64, wp % 2, (2 * cb2 + u)::4, :, :, :].rearrange(
                "p q w c m -> p q w (c m)"),
            in_=kt[64 * u:64 * u + 64, cb2::2, :, :])

    def emit_stitch(wp):
        for u in range(2):
            for cb2 in range(2):
                emit_stitch_kd(wp, u, cb2)
        for cb in range(2):
            for j in range(8):
                emit_stitch_qs(wp, cb, j)

    def emit_norm(w, ws, half, pvn, oacc):
        for nch in range(2):
            pv = pvn[nch]
            pvv = pv.rearrange("p (h s) -> p h s", s=17)
            rcp = rpool.tile([128, 16, 1], F32, tag=f"rcp{nch}",
                             name=f"rcp{nch}")
            nc.vector.reciprocal(rcp[:], pvv[:, :, 16:17])
            rb = rcp[:]
            rbcast = bass.AP(
                tensor=rb.tensor, offset=rb.offset,
                ap=[rb.ap[0], rb.ap[1], [0, 16]],
            )
            nc.vector.tensor_mul(
                oacc[:, nch, 256 * half:256 * half + 256].rearrange(
                    "p (h d) -> p h d", d=16),
                pvv[:, :, 0:16],
                rbcast,
            )

    def emit_compute(wp, ws, stitch_next=False):
        vb = st[wp]["vb"]
        w = 2 * wp + ws
        oacc = opool.tile([128, 2, 512], F32, tag="oacc", name="oacc")
        pvn = None

        for hg in range(8):
            half = hg // 4
            if hg % 4 == 0:
                pvn = [ps_pv.tile([128, 272], F32, tag=f"pv{nch}",
                                  name=f"pv{nch}")
                       for nch in range(2)]
            for sub in range(2):
                ui = ws * 16 + hg * 2 + sub
                ps = ps_s.tile([128, 1024], F32, tag="scores", name="scores")
                for par in range(2):
                    h = 4 * hg + 2 * sub + par
                    for mch in range(2):
                        qq = 2 * par + mch
                        nc.tensor.matmul(
                            ps[:, qq * 256:(qq + 1) * 256],
                            lhsT=KD[:, wp % 2, hg, ws, mch, :],
                            rhs=QS[:, wp % 2, h % 16, h // 16, ws, :],
                            start=True,
                            stop=True,
                        )
                pt = ppool.tile([128, 1024], BF16, tag="pt", name="pt")
                kind = EXP_PATTERN[ui % 32]
                if kind == "A":
                    nc.scalar.activation(
                        pt[:], ps[:], mybir.ActivationFunctionType.Exp,
                        scale=SCALE,
                    )
                else:
                    nc.vector.tensor_scalar(
                        pt[:].bitcast(U16), ps[:], EXP_A, EXP_B,
                        mybir.AluOpType.mult, mybir.AluOpType.add,
                    )
                for par in range(2):
                    h = 4 * hg + 2 * sub + par
                    jj = h % 16
                    for nch in range(2):
                        for mch in range(2):
                            qq = 2 * par + mch
                            nc.tensor.matmul(
                                pvn[nch][:, 17 * jj:17 * jj + 17],
                                lhsT=pt[:, qq * 256 + nch * 128:
                                        qq * 256 + nch * 128 + 128],
                                rhs=vb[(ws, mch)][:, h, :],
                                start=(mch == 0),
                                stop=(mch == 1),
                            )
            if hg % 4 == 3:
                emit_norm(w, ws, half, pvn, oacc)

        nc.sync.dma_start(
            out=out[w].rearrange("(c p) n -> p c n", p=128), in_=oacc[:])

    # software-pipelined pair loop
    emit_load(0, "qk")
    emit_init_a(0)
    emit_load(0, "v")
    emit_init_b(0)
    emit_prep(0)
    emit_init_a(1)
    emit_init_b(1)
    for wp in range(NP):
        if wp + 1 < NP:
            emit_load(wp + 1)
            emit_prep(wp + 1)
        emit_compute(wp, 0)
        emit_compute(wp, 1)
        st.pop(wp - 1, None)


def build_nc():
    nc = bacc.Bacc(
        "TRN2", target_bir_lowering=False, debug=False, num_devices=NCORES
    )
    inp = nc.dram_tensor("inp", [W, N, C], F32, kind="ExternalInput").ap()
    gbias = nc.dram_tensor("gbias", [64, 16, 1024], BF16,
                           kind="ExternalInput").ap()
    uhot = nc.dram_tensor("uhot", [64, 4096], BF16,
                          kind="ExternalInput").ap()
    ident = nc.dram_tensor("ident", [128, 128], F32, kind="ExternalInput").ap()
    out = nc.dram_tensor("out", [W, N, NH * HD], F32,
                         kind="ExternalOutput").ap()
    with tile.TileContext(nc) as tc:
        with ExitStack() as ctx:
            _build_kernel_body(ctx, tc, out, inp, gbias, uhot, ident)
    nc.compile()
    return nc


def _host_consts(table):
    import ml_dtypes
    bf16 = ml_dtypes.bfloat16
    # G'[i, hp, g, w, n] = table[n//4 - i + 63, h]/SCALE, h = hp + 16*g
    j = np.arange(N) // 4
    i0 = np.arange(64)
    idx = j[None, :] - i0[:, None] + 63  # [64, 256]
    g = table[idx]  # [64, 256, NH]
    gb = np.transpose(g, (2, 0, 1)) * np.float32(1.0 / SCALE)  # [NH, 64, 256]
    gbias = np.empty((64, 16, 2, 2, 256), dtype=np.float32)
    for h in range(NH):
        gbias[:, h % 16, h // 16, 0, :] = gb[h]
        gbias[:, h % 16, h // 16, 1, :] = gb[h]
    gbias = gbias.reshape(64, 16, 1024)
    # U[i, qd, w, mch, m] = 1 if (m//4 + 32*mch) == i
    m4 = np.arange(128) // 4
    u = (m4[None, None, :] + 32 * np.arange(2)[None, :, None]
         == np.arange(64)[:, None, None]).astype(np.float32)  # [64, 2, 128]
    uhot = np.broadcast_to(
        u[:, None, None, :, :], (64, 8, 2, 2, 128)).reshape(64, 4096)
    ident = np.eye(128, dtype=np.float32)
    return (np.ascontiguousarray(gbias.astype(bf16)),
            np.ascontiguousarray(uhot.astype(bf16)), ident)


_NC_CACHE = None


def kernel(input, rel_bias_table):
    global _NC_CACHE
    x = np.ascontiguousarray(np.asarray(input, dtype=np.float32))
    tbl = np.asarray(rel_bias_table, dtype=np.float32)
    assert x.shape == (B, N, C), x.shape
    assert tbl.shape == (127, NH), tbl.shape

    if _NC_CACHE is None:
        _NC_CACHE = build_nc()
    nc = _NC_CACHE

    gbias, uhot, ident = _host_consts(tbl)
    in_maps = [
        {
            "inp": np.ascontiguousarray(x[i * W:(i + 1) * W]),
            "gbias": gbias,
            "uhot": uhot,
            "ident": ident,
        }
        for i in range(NCORES)
    ]
    res = run_bass_kernel_spmd(nc, in_maps, list(range(NCORES)))
    return np.concatenate([res.results[i]["out"] for i in range(NCORES)], axis=0)


# revision 36
# speedup vs baseline: 1.0559x; 1.0559x over previous
"""Windowed multi-head self-attention Bass kernel for Trainium2.

Shapes (hardcoded): input [64, 256, 1536] fp32 (packed qkv, 32 heads x 16 dim),
rel_bias_table [127, 32] fp32. Output [64, 256, 512] fp32.

Sharding: data-parallel over the window axis B=64 across 8 NeuronCores
(8 windows per core). Bias tables are preprocessed on host and replicated.

Per-core pipeline (window pairs, software-pipelined):
  - 1 DMA per window loads [128, 2, 1536] fp32 (pair-0 split qk/v so
    transposes start early).
  - PE-transposes q/k sections ([128,128] fp32 blocks) into borrowed
    scores-pool PSUM tiles; DVE copies PSUM->SBUF casting to bf16.
  - Stitched bf16 score matmuls, pair-parity double-buffered: stationary
    KD[:,pr,qd,w,mch,:] holds a 4-head kT quad (rows 0-63) + one-hot U
    rows (64-127, static); moving QS[:,pr,hp,g,w,:] holds qT_h at rows
    16*(h%4) (other q-rows zeroed once at init) + per-head rel-pos bias
    rows G'_h (static).  One matmul per (head, m-chunk) lands
    S^T + bias/scale in PSUM ([128,1024] tiles, 3 in flight).
  - exp split 22:10 per 32 units between ACT (exact exp, scale folded)
    and DVE (Schraudolph exp: bits = round(x*A + B) as uint16, bitcast
    bf16; +-3% rel err, common mode cancels in softmax norm).  GPSIMD
    cannot touch PSUM on TRN2, so Pool only does SBUF work (vb copies)
    and the SWDGE KD stitches.
  - PV reoriented: out[n-chunk, 17] = P^T-chunk.T @ v' (v plus ones
    column -> col 16 = softmax denominator).  17-wide outputs make PV
    ~8x cheaper on PE than the [17,256] orientation and kill the
    output-side transposes.
  - DVE reciprocal of denominators + broadcast-multiply write the
    normalized [n, h*16] output tile; 1 DMA stores each window.
"""

import numpy as np
from contextlib import ExitStack

import concourse.bass as bass
import concourse.bacc as bacc
import concourse.tile as tile
from concourse import mybir
from concourse.bass_utils import run_bass_kernel_spmd

F32 = mybir.dt.float32
F32R = mybir.dt.float32r
BF16 = mybir.dt.bfloat16
U16 = mybir.dt.uint16

NCORES = 8
B = 64
W = B // NCORES
N = 256
C = 1536
NH = 32
HD = 16
SCALE = float(NH) ** -0.5
EXP_A = 128.0 * 1.4426950408889634 * SCALE  # schraudolph mult
EXP_B = 16250.5                             # schraudolph add (calibrated)

# exp engine split: 64 instrs of [128,512] per pair; A=ACT, D=DVE, P=Pool
def _mk_exp_pattern(n_a=22, n_d=10, n_p=0):
    n = n_a + n_d + n_p
    slots = [None] * n
    for kind, cnt in (("A", n_a), ("D", n_d), ("P", n_p)):
        if cnt == 0:
            continue
        for k in range(cnt):
            i = int(k * n / cnt)
            while slots[i % n] is not None:
                i += 1
            slots[i % n] = kind
    return "".join(slots)

EXP_PATTERN = _mk_exp_pattern()
assert len(EXP_PATTERN) == 32


def _build_kernel_body(ctx, tc, out, inp, gbias, uhot, ident):
    nc = tc.nc

    singles = ctx.enter_context(tc.tile_pool(name="singles", bufs=1))
    inpool = ctx.enter_context(tc.tile_pool(name="inpool", bufs=3))
    tpool = ctx.enter_context(tc.tile_pool(name="tpool", bufs=2))
    vpool = ctx.enter_context(tc.tile_pool(name="vpool", bufs=2))
    ppool = ctx.enter_context(tc.tile_pool(name="ppool", bufs=6))
    opool = ctx.enter_context(tc.tile_pool(name="opool", bufs=2))
    rpool = ctx.enter_context(tc.tile_pool(name="rpool", bufs=4))
    ps_s = ctx.enter_context(tc.tile_pool(name="ps_s", bufs=3, space="PSUM"))
    ps_pv = ctx.enter_context(tc.tile_pool(name="ps_pv", bufs=1, space="PSUM"))

    # --- static tiles (DMAs for these are emitted after the first loads) ---
    id_t = singles.tile([128, 128], F32, tag="ident")

    # QS_all[p, pr, hp, g, w, n]: moving operands; rows 0-63 qT (dynamic),
    # rows 64-127 G'_h (static).  Head h = hp + 16*g; pr = pair parity.
    QS = singles.tile([128, 2, 16, 2, 2, 256], BF16, tag="qs", name="qs")
    # KD_all[p, pr, qd, w, mch, m]: stationary; rows 0-63 kT quad (dynamic),
    # rows 64-127 U one-hot (static).
    KD = singles.tile([128, 2, 8, 2, 2, 128], BF16, tag="kd", name="kd")

    def emit_init_a(pr):
        # zero the q-region once: stitches only ever write 16 of each
        # slot's 64 rows; the rest must stay zero for the stitched matmul
        qz = QS[0:64, pr, :, :, :, :]
        nc.scalar.memzero(qz[:, 0:8].rearrange("p h g w n -> p (h g w n)"))
        nc.gpsimd.memset(qz[:, 8:16], 0.0)
        if pr == 0:
            nc.sync.dma_start(out=id_t[:], in_=ident)
        nc.sync.dma_start(
            out=KD[64:128, pr, :, :, :, :].rearrange(
                "p q w c m -> p (q w c m)"),
            in_=uhot)
        nc.sync.dma_start(
            out=QS[64:128, pr, 0:4, :, :, :].rearrange(
                "p h g w n -> p h (g w n)"),
            in_=gbias[:, 0:4, :])

    def emit_init_b(pr):
        for q4 in range(1, 4):
            nc.sync.dma_start(
                out=QS[64:128, pr, 4 * q4:4 * q4 + 4, :, :, :].rearrange(
                    "p h g w n -> p h (g w n)"),
                in_=gbias[:, 4 * q4:4 * q4 + 4, :])

    NP = W // 2
    st = {}

    def emit_load(wp, part=None):
        # part None: both; "qk": cols 0-1024 only; "v": cols 1024-1536
        xin = st.get(wp, {}).get("xin")
        if xin is None:
            xin = {ws: inpool.tile([128, 2, C], F32, tag=f"xin{ws}",
                                   name=f"xin{ws}") for ws in range(2)}
            st[wp] = {"xin": xin}
        lo, hi = {"qk": (0, 1024), "v": (1024, C)}.get(part, (0, C))
        for ws in range(2):
            nc.scalar.dma_start(
                out=xin[ws][:, :, lo:hi],
                in_=inp[2 * wp + ws].rearrange(
                    "(c p) f -> p c f", p=128)[:, :, lo:hi])

    trstate = {"tile": None, "slot": 0}

    def _tr_block(wp, sec, dst, cb):
        xin = st[wp]["xin"]
        for ch in range(2):
            if trstate["slot"] % 4 == 0:
                trstate["tile"] = ps_s.tile([128, 1024], F32, tag="scores",
                                            name="scores")
            sl = trstate["slot"] % 4
            trstate["slot"] += 1
            tr = trstate["tile"][:, sl * 256:(sl + 1) * 256]
            for ws in range(2):
                nc.tensor.transpose(
                    tr[:, ws * 128:(ws + 1) * 128],
                    xin[ws][:, ch, sec + cb * 128: sec + (cb + 1) * 128],
                    id_t[:],
                )
            nc.vector.tensor_copy(
                dst[:, cb, :, ch * 128:(ch + 1) * 128],
                tr.rearrange("p (w n) -> p w n", w=2),
            )

    def emit_prep(wp):
        # transposes + copies + stitch, ordered so stitch sources land first
        xin = st[wp]["xin"]
        qt = tpool.tile([128, 4, 2, 256], BF16, tag="qt", name="qt")
        kt = tpool.tile([128, 4, 2, 256], BF16, tag="kt", name="kt")
        st[wp]["qt"] = qt
        st[wp]["kt"] = kt
        for cbp in range(2):
            for cb in (cbp, cbp + 2):
                _tr_block(wp, 512, kt, cb)
            for u in range(2):
                emit_stitch_kd(wp, u, cbp)
            for cb in (cbp, cbp + 2):
                _tr_block(wp, 0, qt, cb)
            for j in range(8):
                emit_stitch_qs(wp, cbp, j)
        vb = {}
        for ws in range(2):
            for ch in range(2):
                t = vpool.tile([128, NH, 17], BF16, tag=f"vb{ws}{ch}",
                               name=f"vb{ws}{ch}")
                nc.gpsimd.tensor_copy(
                    t[:, :, 0:16],
                    xin[ws][:, ch, 1024:1536].rearrange(
                        "p (h d) -> p h d", d=16),
                )
                if wp < 2:
                    nc.gpsimd.memset(t[:, :, 16:17], 1.0)
                vb[(ws, ch)] = t
        st[wp]["vb"] = vb

    def emit_stitch_qs(wp, cb, j):
        # heads h=8*cb+j and h+16 in one DMA
        qt = st[wp]["qt"]
        t = j % 4
        eng = nc.sync if j % 2 == 0 else nc.gpsimd
        eng.dma_start(
            out=QS[16 * t:16 * t + 16, wp % 2, 8 * cb + j, :, :, :],
            in_=qt[16 * j:16 * j + 16, cb::2, :, :])

    def emit_stitch_kd(wp, u, cb2):
        # quads qd=2*cb2+u and qd+4 in one DMA
        kt = st[wp]["kt"]
        nc.gpsimd.dma_start(
            out=KD[0:64, wp % 2, (2 * cb2 + u)::4, :, :, :].rearrange(
                "p q w c m -> p q w (c m)"),
            in_=kt[64 * u:64 * u + 64, cb2::2, :, :])

    def emit_stitch(wp):
        for u in range(2):
            for cb2 in range(2):
                emit_stitch_kd(wp, u, cb2)
        for cb in range(2):
            for j in range(8):
                emit_stitch_qs(wp, cb, j)

    def emit_norm(w, ws, half, pvn, oacc):
        for nch in range(2):
            pv = pvn[nch]
            pvv = pv.rearrange("p (h s) -> p h s", s=17)
            rcp = rpool.tile([128, 16, 1], F32, tag=f"rcp{nch}",
                             name=f"rcp{nch}")
            nc.vector.reciprocal(rcp[:], pvv[:, :, 16:17])
            rb = rcp[:]
            rbcast = bass.AP(
                tensor=rb.tensor, offset=rb.offset,
                ap=[rb.ap[0], rb.ap[1], [0, 16]],
            )
            nc.vector.tensor_mul(
                oacc[:, nch, 256 * half:256 * half + 256].rearrange(
                    "p (h d) -> p h d", d=16),
                pvv[:, :, 0:16],
                rbcast,
            )

    def emit_compute(wp, ws, stitch_next=False):
        vb = st[wp]["vb"]
        w = 2 * wp + ws
        oacc = opool.tile([128, 2, 512], F32, tag="oacc", name="oacc")
        pvn = None

        for hg in range(8):
            half = hg // 4
            if hg % 4 == 0:
                pvn = [ps_pv.tile([128, 272], F32, tag=f"pv{nch}",
                                  name=f"pv{nch}")
                       for nch in range(2)]
            for sub in range(2):
                ui = ws * 16 + hg * 2 + sub
                ps = ps_s.tile([128, 1024], F32, tag="scores", name="scores")
                for par in range(2):
                    h = 4 * hg + 2 * sub + par
                    for mch in range(2):
                        qq = 2 * par + mch
                        nc.tensor.matmul(
                            ps[:, qq * 256:(qq + 1) * 256],
                            lhsT=KD[:, wp % 2, hg, ws, mch, :],
                            rhs=QS[:, wp % 2, h % 16, h // 16, ws, :],
                            start=True,
                            stop=True,
                        )
                pt = ppool.tile([128, 1024], BF16, tag="pt", name="pt")
                kind = EXP_PATTERN[ui % 32]
                if kind == "A":
                    nc.scalar.activation(
                        pt[:], ps[:], mybir.ActivationFunctionType.Exp,
                        scale=SCALE,
                    )
                else:
                    nc.vector.tensor_scalar(
                        pt[:].bitcast(U16), ps[:], EXP_A, EXP_B,
                        mybir.AluOpType.mult, mybir.AluOpType.add,
                    )
                for par in range(2):
                    h = 4 * hg + 2 * sub + par
                    jj = h % 16
                    for nch in range(2):
                        for mch in range(2):
                            qq = 2 * par + mch
                            nc.tensor.matmul(
                                pvn[nch][:, 17 * jj:17 * jj + 17],
                                lhsT=pt[:, qq * 256 + nch * 128:
                                        qq * 256 + nch * 128 + 128],
                                rhs=vb[(ws, mch)][:, h, :],
                                start=(mch == 0),
                                stop=(mch == 1),
                            )
            if hg % 4 == 3:
                emit_norm(w, ws, half, pvn, oacc)

        nc.sync.dma_start(
            out=out[w].rearrange("(c p) n -> p c n", p=128), in_=oacc[:])

    # software-pipelined pair loop
    emit_load(0, "qk")
    emit_init_a(0)
    emit_load(0, "v")
    emit_init_b(0)
    emit_prep(0)
    emit_init_a(1)
    emit_init_b(1)
    for wp in range(NP):
        if wp + 1 < NP:
            emit_load(wp + 1)
            emit_prep(wp + 1)
        emit_compute(wp, 0)
        emit_compute(wp, 1)
        st.pop(wp - 1, None)


def build_nc():
    nc = bacc.Bacc(
        "TRN2", target_bir_lowering=False, debug=False, num_devices=NCORES
    )
    inp = nc.dram_tensor("inp", [W, N, C], F32, kind="ExternalInput").ap()
    gbias = nc.dram_tensor("gbias", [64, 16, 1024], BF16,
                           kind="ExternalInput").ap()
    uhot = nc.dram_tensor("uhot", [64, 4096], BF16,
                          kind="ExternalInput").ap()
    ident = nc.dram_tensor("ident", [128, 128], F32, kind="ExternalInput").ap()
    out = nc.dram_tensor("out", [W, N, NH * HD], F32,
                         kind="ExternalOutput").ap()
    with tile.TileContext(nc) as tc:
        with ExitStack() as ctx:
            _build_kernel_body(ctx, tc, out, inp, gbias, uhot, ident)
    nc.compile()
    return nc


def _host_consts(table):
    import ml_dtypes
    bf16 = ml_dtypes.bfloat16
    # G'[i, hp, g, w, n] = table[n//4 - i + 63, h]/SCALE, h = hp + 16*g
    j = np.arange(N) // 4
    i0 = np.arange(64)
    idx = j[None, :] - i0[:, None] + 63  # [64, 256]
    g = table[idx]  # [64, 256, NH]
    gb = np.transpose(g, (2, 0, 1)) * np.float32(1.0 / SCALE)  # [NH, 64, 256]
    gbias = np.empty((64, 16, 2, 2, 256), dtype=np.float32)
    for h in range(NH):
        gbias[:, h % 16, h // 16, 0, :] = gb[h]
        gbias[:, h % 16, h // 16, 1, :] = gb[h]
    gbias = gbias.reshape(64, 16, 1024)
    # U[i, qd, w, mch, m] = 1 if (m//4 + 32*mch) == i
    m4 = np.arange(128) // 4
    u = (m4[None, None, :] + 32 * np.arange(2)[None, :, None]
         == np.arange(64)[:, None, None]).astype(np.float32)  # [64, 2, 128]
    uhot = np.broadcast_to(
        u[:, None, None, :, :], (64, 8, 2, 2, 128)).reshape(64, 4096)
    ident = np.eye(128, dtype=np.float32)
    return (np.ascontiguousarray(gbias.astype(bf16)),
            np.ascontiguousarray(uhot.astype(bf16)), ident)


_NC_CACHE = None


def kernel(input, rel_bias_table):
    global _NC_CACHE
    x = np.ascontiguousarray(np.asarray(input, dtype=np.float32))
    tbl = np.asarray(rel_bias_table, dtype=np.float32)
    assert x.shape == (B, N, C), x.shape
    assert tbl.shape == (127, NH), tbl.shape

    if _NC_CACHE is None:
        _NC_CACHE = build_nc()
    nc = _NC_CACHE

    gbias, uhot, ident = _host_consts(tbl)
    in_maps = [
        {
            "inp": np.ascontiguousarray(x[i * W:(i + 1) * W]),
            "gbias": gbias,
            "uhot": uhot,
            "ident": ident,
        }
        for i in range(NCORES)
    ]
    res = run_bass_kernel_spmd(nc, in_maps, list(range(NCORES)))
    return np.concatenate([res.results[i]["out"] for i in range(NCORES)], axis=0)
